# revision 10
# baseline (speedup 1.0000x reference)
"""Multi-head causal attention (B=2, S=4096, DM=768, H=12) on 8 trn2 cores.

Sharding: core c handles batch c//4, heads 3*(c%4) .. +3 (data + head parallel).
Each core computes QKV projection for its 3 heads, causal flash-style attention,
and a partial output projection (contraction over its 192 out-dims). Host sums
the 4 partials per batch; qkv/out biases are folded in exactly on the host.

On-chip layout: Q,K are kept transposed [dh, S] (computed via PE-transpose of x
feeding the QKV matmul); scores are computed as S^T blocks [sk=128, sq=512] so
softmax needs no partition-dim reductions: the running denominator comes from an
appended ones-column in V, and the final divide is broadcast with a K=1 matmul.
No max-subtraction is needed (scores ~ N(0,1) in fp32). All matmuls use fp32r.
"""
import sys

sys.path.insert(0, "/opt/trn_rl_repo")
import numpy as np

B, S, DM, H, DH = 2, 4096, 768, 12, 64
NCORES = 8
QB = 512          # q-block (psum free dim)
KB = 128          # k-block (psum partition dim)
NQB = S // QB     # 8
RC = 512          # row chunk for the projection phase
KCH = DM // 128   # 6 contraction chunks of the model dim

_cache = {}


def _build():
    import concourse.mybir as mybir
    import concourse.tile as tile
    from concourse import bacc
    from contextlib import ExitStack

    f32 = mybir.dt.float32
    fr = mybir.dt.float32r

    nc = bacc.Bacc("TRN2", target_bir_lowering=False, debug=False)
    xb = nc.dram_tensor("xb", [S, DM], f32, kind="ExternalInput")
    wt = nc.dram_tensor("wt", [DM, 512], f32, kind="ExternalInput")
    wv = nc.dram_tensor("wv", [DM, 192], f32, kind="ExternalInput")
    wo = nc.dram_tensor("wo", [192, DM], f32, kind="ExternalInput")
    mk = nc.dram_tensor("mk", [128, 896], f32, kind="ExternalInput")
    bqk = nc.dram_tensor("bqk", [128, 4], f32, kind="ExternalInput")
    id128 = nc.dram_tensor("id128", [128, 128], f32, kind="ExternalInput")
    vones = nc.dram_tensor("vones", [128, 97], f32, kind="ExternalInput")
    z = nc.dram_tensor("z", [S, DM], f32, kind="ExternalOutput")

    with tile.TileContext(nc) as tc, ExitStack() as st:
        pers = st.enter_context(tc.tile_pool(name="pers", bufs=1))
        # 0: Q^T heads 0|1 packed in partition halves, 1: Q^T head2 duplicated,
        # 2: K^T heads 0|1, 3: K^T head2 duplicated
        QK = [
            pers.tile([128, S], fr, tag=f"qk{i}", name=f"qk{i}") for i in range(4)
        ]
        # V natural layout + ones column per head: [sk%128, sk//128, 3*(64+1)]
        V3 = pers.tile([128, S // 128, 3 * (DH + 1)], fr)
        wt_sb = pers.tile([128, KCH, 512], fr)
        wv_sb = pers.tile([128, KCH, 192], fr)
        wo1_sb = pers.tile([128, DM], fr)
        wo2_sb = pers.tile([64, DM], fr)
        mask_sb = pers.tile([128, 896], fr)
        bqk_sb = pers.tile([128, 4], f32)
        ident = pers.tile([128, 128], fr)
        ones_sb = pers.tile([1, 64], fr)

        nc.sync.dma_start(out=wt_sb, in_=wt.ap().bitcast(fr).rearrange("(h p) c -> p h c", p=128))
        nc.sync.dma_start(out=wv_sb, in_=wv.ap().bitcast(fr).rearrange("(h p) c -> p h c", p=128))
        nc.sync.dma_start(out=wo1_sb, in_=wo.ap().bitcast(fr)[0:128, :])
        nc.sync.dma_start(out=wo2_sb, in_=wo.ap().bitcast(fr)[128:192, :])
        nc.sync.dma_start(out=mask_sb, in_=mk.ap().bitcast(fr))
        nc.sync.dma_start(out=bqk_sb, in_=bqk.ap())
        nc.sync.dma_start(out=ident, in_=id128.ap().bitcast(fr))
        nc.sync.dma_start(out=ones_sb, in_=vones.ap().bitcast(fr)[0:1, 0:64])
        v3v = V3.rearrange("p c (h e) -> p c h e", e=DH + 1)
        nc.sync.dma_start(
            out=v3v[:, :, :, DH : DH + 1],
            in_=vones.ap().bitcast(fr)[:, 1:97].rearrange("p (c h) -> p c h", h=3)[
                :, :, :
            ].unsqueeze(3),
        )

        # ---- Phase 1+2: transpose x by row-chunks and project QKV ----
        with (
            tc.tile_pool(name="xs", bufs=2) as xsp,
            tc.tile_pool(name="xt", bufs=2) as xtp,
            tc.tile_pool(name="pst", bufs=2, space="PSUM") as pst,
            tc.tile_pool(name="psq", bufs=2, space="PSUM") as psq,
            tc.tile_pool(name="psv", bufs=2, space="PSUM") as psv,
        ):
            for r in range(S // RC):
                xs = xsp.tile([128, RC // 128, DM], fr)
                nc.sync.dma_start(
                    out=xs,
                    in_=xb.ap().bitcast(fr)[r * RC : (r + 1) * RC, :].rearrange(
                        "(c p) d -> p c d", p=128
                    ),
                )
                xt = xtp.tile([128, KCH, RC], fr)
                for k in range(KCH):
                    for c in range(RC // 128):
                        pt = pst.tile([128, 128], fr)
                        nc.tensor.transpose(
                            pt,
                            xs[:, c, 128 * k : 128 * (k + 1)],
                            ident,
                        )
                        nc.vector.tensor_copy(xt[:, k, 128 * c : 128 * (c + 1)], pt)
                # transposed Q/K products: 4 column chunks of 128
                for ch in range(4):
                    pq = psq.tile([128, RC], f32)
                    for k in range(KCH):
                        nc.tensor.matmul(
                            pq,
                            wt_sb[:, k, 128 * ch : 128 * (ch + 1)],
                            xt[:, k, :],
                            start=(k == 0),
                            stop=(k == KCH - 1),
                        )
                    nc.vector.tensor_scalar_add(
                        QK[ch][:, r * RC : (r + 1) * RC], pq, bqk_sb[:, ch : ch + 1]
                    )
                # V natural: rows on partitions
                for c in range(RC // 128):
                    pv = psv.tile([128, 192], f32)
                    for k in range(KCH):
                        nc.tensor.matmul(
                            pv,
                            xt[:, k, 128 * c : 128 * (c + 1)],
                            wv_sb[:, k, :],
                            start=(k == 0),
                            stop=(k == KCH - 1),
                        )
                    sidx = r * (RC // 128) + c
                    dst = V3[:, sidx : sidx + 1, :].rearrange(
                        "p c (h e) -> p (c h) e", e=DH + 1
                    )[:, :, 0:DH]
                    nc.vector.tensor_copy(dst, pv.rearrange("p (h e) -> p h e", e=DH))

        # ---- Phase 3: causal attention + partial out-projection ----
        with (
            tc.tile_pool(name="ptp", bufs=6) as ptp,
            tc.tile_pool(name="atp", bufs=2) as atp,
            tc.tile_pool(name="rcp", bufs=4) as rcp,
            tc.tile_pool(name="zsp", bufs=4) as zsp,
            tc.tile_pool(name="psS", bufs=3, space="PSUM") as psS,
            tc.tile_pool(name="psO", bufs=1, space="PSUM") as psO,
            tc.tile_pool(name="psZ", bufs=2, space="PSUM") as psZ,
        ):
            for j in range(NQB):
                nkb = 4 * j + 4
                qs = slice(j * QB, (j + 1) * QB)
                os_ = [
                    psO.tile([DH + 1, QB], f32, tag=f"o{h}", name=f"o{h}")
                    for h in range(3)
                ]
                for i in range(nkb):
                    ks = slice(128 * i, 128 * (i + 1))
                    half = 64 * (i % 2)
                    srcs = [
                        (QK[2][0:64, ks], QK[0][0:64, qs]),
                        (QK[2][64:128, ks], QK[0][64:128, qs]),
                        (QK[3][half : half + 64, ks], QK[1][half : half + 64, qs]),
                    ]
                    for h, (kt, qt) in enumerate(srcs):
                        ps = psS.tile([128, QB], f32, tag="s")
                        nc.tensor.matmul(
                            ps, kt, qt, start=True, stop=True
                        )
                        pt_t = ptp.tile([128, QB], fr, tag="pt")
                        nc.scalar.activation(
                            pt_t, ps, mybir.ActivationFunctionType.Exp, scale=DH**-0.5
                        )
                        if i >= 4 * j:  # diagonal block: causal mask
                            o = 128 * i - QB * j
                            w = o + 128
                            nc.vector.tensor_mul(
                                pt_t[:, 0:w],
                                pt_t[:, 0:w],
                                mask_sb[:, 384 - o : 384 - o + w],
                            )
                        vsl = V3[:, i : i + 1, 65 * h : 65 * h + 65].rearrange(
                            "p a b -> p (a b)"
                        )
                        nc.tensor.matmul(
                            os_[h],
                            vsl,
                            pt_t,
                            start=(i == 0),
                            stop=(i == nkb - 1),
                        )
                # normalize: divide by the ones-column accumulation
                at12 = atp.tile([128, QB], tag="a12", dtype=fr)
                at2 = atp.tile([64, QB], tag="a2", dtype=fr)
                dsts = [at12[0:64, :], at12[64:128, :], at2[0:64, :]]
                for h in range(3):
                    rc1 = rcp.tile([1, QB], fr)
                    with nc.allow_low_precision(reason="fp32r softmax denom"):
                        nc.vector.reciprocal(rc1, os_[h][DH : DH + 1, :])
                    pb = psZ.tile([64, QB], f32, tag="zb")
                    nc.tensor.matmul(
                        pb, ones_sb, rc1, start=True, stop=True
                    )
                    bb = rcp.tile([64, QB], f32, tag="bb")
                    nc.vector.tensor_copy(bb, pb)
                    nc.vector.tensor_mul(dsts[h], os_[h][0:DH, :], bb)
                # partial out-projection for this q-block
                for m in range(QB // 128):
                    for n0, nw in ((0, 512), (512, 256)):
                        pz = psZ.tile([128, nw], f32, tag="zb")
                        nc.tensor.matmul(
                            pz,
                            at12[:, 128 * m : 128 * (m + 1)],
                            wo1_sb[:, n0 : n0 + nw],
                            start=True,
                            stop=False,
                        )
                        nc.tensor.matmul(
                            pz,
                            at2[:, 128 * m : 128 * (m + 1)],
                            wo2_sb[:, n0 : n0 + nw],
                            start=False,
                            stop=True,
                        )
                        zs = zsp.tile([128, nw], f32, tag="zs")
                        nc.vector.tensor_copy(zs, pz)
                        r0 = j * QB + 128 * m
                        nc.sync.dma_start(
                            out=z.ap()[r0 : r0 + 128, n0 : n0 + nw], in_=zs
                        )

    nc.compile()
    return nc


def _core_inputs(c, x, w_qkv, b_qkv, w_out, mk):
    b = c // 4
    h0 = 3 * (c % 4)

    def cols(mat_idx, h):
        base = mat_idx * DM + h * DH
        return w_qkv[:, base : base + DH]

    wt = np.ascontiguousarray(
        np.concatenate(
            [cols(0, h0), cols(0, h0 + 1), cols(0, h0 + 2), cols(0, h0 + 2),
             cols(1, h0), cols(1, h0 + 1), cols(1, h0 + 2), cols(1, h0 + 2)],
            axis=1,
        )
    )
    wv = np.ascontiguousarray(
        np.concatenate([cols(2, h0), cols(2, h0 + 1), cols(2, h0 + 2)], axis=1)
    )
    wo = np.ascontiguousarray(w_out[h0 * DH : h0 * DH + 192, :])

    def bias(mat_idx, h):
        base = mat_idx * DM + h * DH
        return b_qkv[base : base + DH]

    bqk = np.stack(
        [
            np.concatenate([bias(0, h0), bias(0, h0 + 1)]),
            np.concatenate([bias(0, h0 + 2), bias(0, h0 + 2)]),
            np.concatenate([bias(1, h0), bias(1, h0 + 1)]),
            np.concatenate([bias(1, h0 + 2), bias(1, h0 + 2)]),
        ],
        axis=1,
    ).astype(np.float32)
    return {
        "xb": np.ascontiguousarray(x[b]),
        "wt": wt,
        "wv": wv,
        "wo": wo,
        "mk": mk,
        "bqk": np.ascontiguousarray(bqk),
        "id128": np.eye(128, dtype=np.float32),
        "vones": np.ones((128, 97), dtype=np.float32),
    }


def _run(inputs, trace=False):
    from concourse.bass_utils import run_bass_kernel_spmd

    x = np.asarray(inputs["x"], dtype=np.float32)
    w_qkv = np.asarray(inputs["w_qkv"], dtype=np.float32)
    b_qkv = np.asarray(inputs["b_qkv"], dtype=np.float32)
    w_out = np.asarray(inputs["w_out"], dtype=np.float32)
    b_out = np.asarray(inputs["b_out"], dtype=np.float32)

    if "nc" not in _cache:
        _cache["nc"] = _build()
    nc = _cache["nc"]

    mk = (
        np.arange(896, dtype=np.int64)[None, :]
        >= (np.arange(128, dtype=np.int64)[:, None] + 384)
    ).astype(np.float32)
    in_maps = [_core_inputs(c, x, w_qkv, b_qkv, w_out, mk) for c in range(NCORES)]
    res = run_bass_kernel_spmd(
        nc, in_maps, core_ids=list(range(NCORES)), trace=trace
    )

    out = np.zeros((B, S, DM), dtype=np.float32)
    for c in range(NCORES):
        out[c // 4] += res.results[c]["z"]
    out += b_qkv[2 * DM : 3 * DM] @ w_out + b_out
    return out, res


def kernel(**inputs):
    out, _ = _run(inputs, trace=False)
    return out


# revision 11
# speedup vs baseline: 1.8468x; 1.8468x over previous
"""Multi-head causal attention (B=2, S=4096, DM=768, H=12) on 8 trn2 cores.

Sharding: core c handles batch c//4, heads 3*(c%4) .. +3 (data + head parallel).
Each core computes QKV projection for its 3 heads, causal flash-style attention,
and a partial output projection (contraction over its 192 out-dims). Host sums
the 4 partials per batch; qkv/out biases are folded in exactly on the host.

On-chip layout: Q,K are kept transposed [dh, S] (computed via PE-transpose of x
feeding the QKV matmul); scores are computed as S^T blocks [sk=128, sq=512] so
softmax needs no partition-dim reductions: the running denominator comes from an
appended ones-column in V, and the final divide is broadcast with a K=1 matmul.
No max-subtraction is needed (scores ~ N(0,1) in fp32). All matmuls use fp32r.
"""
import sys

sys.path.insert(0, "/opt/trn_rl_repo")
import numpy as np

B, S, DM, H, DH = 2, 4096, 768, 12, 64
NCORES = 8
QB = 512          # q-block (psum free dim)
KB = 128          # k-block (psum partition dim)
NQB = S // QB     # 8
RC = 512          # row chunk for the projection phase
KCH = DM // 128   # 6 contraction chunks of the model dim

_cache = {}


def _build(loop_n=None):
    import concourse.mybir as mybir
    import concourse.tile as tile
    from concourse import bacc
    from contextlib import ExitStack

    f32 = mybir.dt.float32
    fr = mybir.dt.float32r

    nc = bacc.Bacc("TRN2", target_bir_lowering=False, debug=False)
    xb = nc.dram_tensor("xb", [S, DM], f32, kind="ExternalInput")
    wt = nc.dram_tensor("wt", [DM, 512], f32, kind="ExternalInput")
    wv = nc.dram_tensor("wv", [DM, 192], f32, kind="ExternalInput")
    wo = nc.dram_tensor("wo", [192, DM], f32, kind="ExternalInput")
    mk = nc.dram_tensor("mk", [128, 896], f32, kind="ExternalInput")
    bqk = nc.dram_tensor("bqk", [128, 4], f32, kind="ExternalInput")
    id128 = nc.dram_tensor("id128", [128, 128], f32, kind="ExternalInput")
    vones = nc.dram_tensor("vones", [128, 97], f32, kind="ExternalInput")
    z = nc.dram_tensor("z", [S, DM], f32, kind="ExternalOutput")

    with tile.TileContext(nc) as tc, ExitStack() as st:
        if loop_n is not None:
            st.enter_context(tc.For_i(0, loop_n, 1))
        pers = st.enter_context(tc.tile_pool(name="pers", bufs=1))
        # 0: Q^T heads 0|1 packed in partition halves, 1: Q^T head2 duplicated,
        # 2: K^T heads 0|1, 3: K^T head2 duplicated
        QK = [
            pers.tile([128, S], fr, tag=f"qk{i}", name=f"qk{i}") for i in range(4)
        ]
        # V natural layout + ones column per head: [sk%128, sk//128, 3*(64+1)]
        V3 = pers.tile([128, S // 128, 3 * (DH + 1)], fr)
        wt_sb = pers.tile([128, KCH, 512], fr)
        wv_sb = pers.tile([128, KCH, 192], fr)
        wo1_sb = pers.tile([128, DM], fr)
        wo2_sb = pers.tile([64, DM], fr)
        mask_sb = pers.tile([128, 896], fr)
        bqk_sb = pers.tile([128, 4], f32)
        ident = pers.tile([128, 128], fr)
        ones_sb = pers.tile([1, 64], fr)

        nc.sync.dma_start(out=wt_sb, in_=wt.ap().bitcast(fr).rearrange("(h p) c -> p h c", p=128))
        nc.sync.dma_start(out=wv_sb, in_=wv.ap().bitcast(fr).rearrange("(h p) c -> p h c", p=128))
        nc.sync.dma_start(out=wo1_sb, in_=wo.ap().bitcast(fr)[0:128, :])
        nc.sync.dma_start(out=wo2_sb, in_=wo.ap().bitcast(fr)[128:192, :])
        nc.sync.dma_start(out=mask_sb, in_=mk.ap().bitcast(fr))
        nc.sync.dma_start(out=bqk_sb, in_=bqk.ap())
        nc.sync.dma_start(out=ident, in_=id128.ap().bitcast(fr))
        nc.sync.dma_start(out=ones_sb, in_=vones.ap().bitcast(fr)[0:1, 0:64])
        v3v = V3.rearrange("p c (h e) -> p c h e", e=DH + 1)
        nc.sync.dma_start(
            out=v3v[:, :, :, DH : DH + 1],
            in_=vones.ap().bitcast(fr)[:, 1:97].rearrange("p (c h) -> p c h", h=3)[
                :, :, :
            ].unsqueeze(3),
        )

        # ---- Phase 1+2: transpose x by row-chunks and project QKV ----
        with (
            tc.tile_pool(name="xs", bufs=2) as xsp,
            tc.tile_pool(name="xt", bufs=2) as xtp,
            tc.tile_pool(name="pst", bufs=2, space="PSUM") as pst,
            tc.tile_pool(name="psq", bufs=2, space="PSUM") as psq,
            tc.tile_pool(name="psv", bufs=2, space="PSUM") as psv,
        ):
            for r in range(S // RC):
                xs = xsp.tile([128, RC // 128, DM], fr)
                nc.sync.dma_start(
                    out=xs,
                    in_=xb.ap().bitcast(fr)[r * RC : (r + 1) * RC, :].rearrange(
                        "(c p) d -> p c d", p=128
                    ),
                )
                xt = xtp.tile([128, KCH, RC], fr)
                for k in range(KCH):
                    for c in range(RC // 128):
                        pt = pst.tile([128, 128], fr)
                        nc.tensor.transpose(
                            pt,
                            xs[:, c, 128 * k : 128 * (k + 1)],
                            ident,
                        )
                        nc.vector.tensor_copy(xt[:, k, 128 * c : 128 * (c + 1)], pt)
                # transposed Q/K products: 4 column chunks of 128
                for ch in range(4):
                    pq = psq.tile([128, RC], f32)
                    for k in range(KCH):
                        nc.tensor.matmul(
                            pq,
                            wt_sb[:, k, 128 * ch : 128 * (ch + 1)],
                            xt[:, k, :],
                            start=(k == 0),
                            stop=(k == KCH - 1),
                        )
                    nc.vector.tensor_scalar_add(
                        QK[ch][:, r * RC : (r + 1) * RC], pq, bqk_sb[:, ch : ch + 1]
                    )
                # V natural: rows on partitions
                for c in range(RC // 128):
                    pv = psv.tile([128, 192], f32)
                    for k in range(KCH):
                        nc.tensor.matmul(
                            pv,
                            xt[:, k, 128 * c : 128 * (c + 1)],
                            wv_sb[:, k, :],
                            start=(k == 0),
                            stop=(k == KCH - 1),
                        )
                    sidx = r * (RC // 128) + c
                    dst = V3[:, sidx : sidx + 1, :].rearrange(
                        "p c (h e) -> p (c h) e", e=DH + 1
                    )[:, :, 0:DH]
                    nc.vector.tensor_copy(dst, pv.rearrange("p (h e) -> p h e", e=DH))

        # ---- Phase 3: causal attention + partial out-projection ----
        with (
            tc.tile_pool(name="ptp", bufs=6) as ptp,
            tc.tile_pool(name="atp", bufs=2) as atp,
            tc.tile_pool(name="rcp", bufs=4) as rcp,
            tc.tile_pool(name="zsp", bufs=4) as zsp,
            tc.tile_pool(name="psS", bufs=3, space="PSUM") as psS,
            tc.tile_pool(name="psO", bufs=1, space="PSUM") as psO,
            tc.tile_pool(name="psZ", bufs=2, space="PSUM") as psZ,
        ):
            for j in range(NQB):
                nkb = 4 * j + 4
                qs = slice(j * QB, (j + 1) * QB)
                os_ = [
                    psO.tile([DH + 1, QB], f32, tag=f"o{h}", name=f"o{h}")
                    for h in range(3)
                ]
                for i in range(nkb):
                    ks = slice(128 * i, 128 * (i + 1))
                    half = 64 * (i % 2)
                    srcs = [
                        (QK[2][0:64, ks], QK[0][0:64, qs]),
                        (QK[2][64:128, ks], QK[0][64:128, qs]),
                        (QK[3][half : half + 64, ks], QK[1][half : half + 64, qs]),
                    ]
                    for h, (kt, qt) in enumerate(srcs):
                        ps = psS.tile([128, QB], f32, tag="s")
                        nc.tensor.matmul(
                            ps, kt, qt, start=True, stop=True
                        )
                        pt_t = ptp.tile([128, QB], fr, tag="pt")
                        nc.scalar.activation(
                            pt_t, ps, mybir.ActivationFunctionType.Exp, scale=DH**-0.5
                        )
                        if i >= 4 * j:  # diagonal block: causal mask
                            o = 128 * i - QB * j
                            w = o + 128
                            nc.vector.tensor_mul(
                                pt_t[:, 0:w],
                                pt_t[:, 0:w],
                                mask_sb[:, 384 - o : 384 - o + w],
                            )
                        vsl = V3[:, i : i + 1, 65 * h : 65 * h + 65].rearrange(
                            "p a b -> p (a b)"
                        )
                        nc.tensor.matmul(
                            os_[h],
                            vsl,
                            pt_t,
                            start=(i == 0),
                            stop=(i == nkb - 1),
                        )
                # normalize: divide by the ones-column accumulation
                at12 = atp.tile([128, QB], tag="a12", dtype=fr)
                at2 = atp.tile([64, QB], tag="a2", dtype=fr)
                dsts = [at12[0:64, :], at12[64:128, :], at2[0:64, :]]
                for h in range(3):
                    rc1 = rcp.tile([1, QB], fr)
                    with nc.allow_low_precision(reason="fp32r softmax denom"):
                        nc.vector.reciprocal(rc1, os_[h][DH : DH + 1, :])
                    pb = psZ.tile([64, QB], f32, tag="zb")
                    nc.tensor.matmul(
                        pb, ones_sb, rc1, start=True, stop=True
                    )
                    bb = rcp.tile([64, QB], f32, tag="bb")
                    nc.vector.tensor_copy(bb, pb)
                    nc.vector.tensor_mul(dsts[h], os_[h][0:DH, :], bb)
                # partial out-projection for this q-block
                for m in range(QB // 128):
                    for n0, nw in ((0, 512), (512, 256)):
                        pz = psZ.tile([128, nw], f32, tag="zb")
                        nc.tensor.matmul(
                            pz,
                            at12[:, 128 * m : 128 * (m + 1)],
                            wo1_sb[:, n0 : n0 + nw],
                            start=True,
                            stop=False,
                        )
                        nc.tensor.matmul(
                            pz,
                            at2[:, 128 * m : 128 * (m + 1)],
                            wo2_sb[:, n0 : n0 + nw],
                            start=False,
                            stop=True,
                        )
                        zs = zsp.tile([128, nw], f32, tag="zs")
                        nc.vector.tensor_copy(zs, pz)
                        r0 = j * QB + 128 * m
                        nc.sync.dma_start(
                            out=z.ap()[r0 : r0 + 128, n0 : n0 + nw], in_=zs
                        )

    nc.compile()
    return nc


def _core_inputs(c, x, w_qkv, b_qkv, w_out, mk):
    b = c // 4
    h0 = 3 * (c % 4)

    def cols(mat_idx, h):
        base = mat_idx * DM + h * DH
        return w_qkv[:, base : base + DH]

    wt = np.ascontiguousarray(
        np.concatenate(
            [cols(0, h0), cols(0, h0 + 1), cols(0, h0 + 2), cols(0, h0 + 2),
             cols(1, h0), cols(1, h0 + 1), cols(1, h0 + 2), cols(1, h0 + 2)],
            axis=1,
        )
    )
    wv = np.ascontiguousarray(
        np.concatenate([cols(2, h0), cols(2, h0 + 1), cols(2, h0 + 2)], axis=1)
    )
    wo = np.ascontiguousarray(w_out[h0 * DH : h0 * DH + 192, :])

    def bias(mat_idx, h):
        base = mat_idx * DM + h * DH
        return b_qkv[base : base + DH]

    bqk = np.stack(
        [
            np.concatenate([bias(0, h0), bias(0, h0 + 1)]),
            np.concatenate([bias(0, h0 + 2), bias(0, h0 + 2)]),
            np.concatenate([bias(1, h0), bias(1, h0 + 1)]),
            np.concatenate([bias(1, h0 + 2), bias(1, h0 + 2)]),
        ],
        axis=1,
    ).astype(np.float32)
    return {
        "xb": np.ascontiguousarray(x[b]),
        "wt": wt,
        "wv": wv,
        "wo": wo,
        "mk": mk,
        "bqk": np.ascontiguousarray(bqk),
        "id128": np.eye(128, dtype=np.float32),
        "vones": np.ones((128, 97), dtype=np.float32),
    }


def _run(inputs, trace=False):
    from concourse.bass_utils import run_bass_kernel_spmd

    x = np.asarray(inputs["x"], dtype=np.float32)
    w_qkv = np.asarray(inputs["w_qkv"], dtype=np.float32)
    b_qkv = np.asarray(inputs["b_qkv"], dtype=np.float32)
    w_out = np.asarray(inputs["w_out"], dtype=np.float32)
    b_out = np.asarray(inputs["b_out"], dtype=np.float32)

    if "nc" not in _cache:
        _cache["nc"] = _build()
    nc = _cache["nc"]

    mk = (
        np.arange(896, dtype=np.int64)[None, :]
        >= (np.arange(128, dtype=np.int64)[:, None] + 384)
    ).astype(np.float32)
    in_maps = [_core_inputs(c, x, w_qkv, b_qkv, w_out, mk) for c in range(NCORES)]
    res = run_bass_kernel_spmd(
        nc, in_maps, core_ids=list(range(NCORES)), trace=trace
    )

    out = np.zeros((B, S, DM), dtype=np.float32)
    for c in range(NCORES):
        out[c // 4] += res.results[c]["z"]
    out += b_qkv[2 * DM : 3 * DM] @ w_out + b_out
    return out, res


def kernel(**inputs):
    out, _ = _run(inputs, trace=False)
    return out


# revision 14
# speedup vs baseline: 2.2055x; 1.1942x over previous
"""Multi-head causal attention (B=2, S=4096, DM=768, H=12) on 8 trn2 cores.

Sharding: core c handles batch c//4, heads 3*(c%4) .. +3 (data + head parallel).
Each core computes QKV projection for its 3 heads, causal flash-style attention,
and a partial output projection (contraction over its 192 out-dims). Host sums
the 4 partials per batch; qkv/out biases are folded in exactly on the host.

On-chip layout: Q,K are kept transposed [dh, S] (computed via PE-transpose of x
feeding the QKV matmul); scores are computed as S^T blocks [sk=128, sq=512] so
softmax needs no partition-dim reductions: the running denominator comes from an
appended ones-column in V, and the final divide is broadcast with a K=1 matmul.
No max-subtraction is needed (scores ~ N(0,1) in fp32). All matmuls use fp32r.
"""
import sys

sys.path.insert(0, "/opt/trn_rl_repo")
import numpy as np

B, S, DM, H, DH = 2, 4096, 768, 12, 64
NCORES = 8
QB = 512          # q-block (psum free dim)
KB = 128          # k-block (psum partition dim)
NQB = S // QB     # 8
RC = 512          # row chunk for the projection phase
KCH = DM // 128   # 6 contraction chunks of the model dim

_cache = {}


def _build(loop_n=None, stages=frozenset({'p12','attn','norm','oproj'})):
    import concourse.mybir as mybir
    import concourse.tile as tile
    from concourse import bacc
    from contextlib import ExitStack

    f32 = mybir.dt.float32
    fr = mybir.dt.float32r

    nc = bacc.Bacc("TRN2", target_bir_lowering=False, debug=False)
    xb = nc.dram_tensor("xb", [S, DM], f32, kind="ExternalInput")
    wt = nc.dram_tensor("wt", [DM, 512], f32, kind="ExternalInput")
    wv = nc.dram_tensor("wv", [DM, 192], f32, kind="ExternalInput")
    wo = nc.dram_tensor("wo", [192, DM], f32, kind="ExternalInput")
    mk = nc.dram_tensor("mk", [128, 896], f32, kind="ExternalInput")
    bqk = nc.dram_tensor("bqk", [128, 4], f32, kind="ExternalInput")
    id128 = nc.dram_tensor("id128", [128, 128], f32, kind="ExternalInput")
    vones = nc.dram_tensor("vones", [128, 97], f32, kind="ExternalInput")
    z = nc.dram_tensor("z", [S, DM], f32, kind="ExternalOutput")

    with tile.TileContext(nc) as tc, ExitStack() as st:
        if loop_n is not None:
            st.enter_context(tc.For_i(0, loop_n, 1))
        pers = st.enter_context(tc.tile_pool(name="pers", bufs=1))
        # 0: Q^T heads 0|1 packed in partition halves, 1: Q^T head2 duplicated,
        # 2: K^T heads 0|1, 3: K^T head2 duplicated
        QK = [
            pers.tile([128, S], fr, tag=f"qk{i}", name=f"qk{i}") for i in range(4)
        ]
        # V natural layout + ones column per head: [sk%128, sk//128, 3*(64+1)]
        V3 = pers.tile([128, S // 128, 3 * (DH + 1)], fr)
        wt_sb = pers.tile([128, KCH, 512], fr)
        wv_sb = pers.tile([128, KCH, 192], fr)
        wo1_sb = pers.tile([128, DM], fr)
        wo2_sb = pers.tile([64, DM], fr)
        mask_sb = pers.tile([128, 896], fr)
        bqk_sb = pers.tile([128, 4], f32)
        ident = pers.tile([128, 128], fr)
        ones_sb = pers.tile([1, 64], fr)

        nc.sync.dma_start(out=wt_sb, in_=wt.ap().bitcast(fr).rearrange("(h p) c -> p h c", p=128))
        nc.sync.dma_start(out=wv_sb, in_=wv.ap().bitcast(fr).rearrange("(h p) c -> p h c", p=128))
        nc.sync.dma_start(out=wo1_sb, in_=wo.ap().bitcast(fr)[0:128, :])
        nc.sync.dma_start(out=wo2_sb, in_=wo.ap().bitcast(fr)[128:192, :])
        nc.sync.dma_start(out=mask_sb, in_=mk.ap().bitcast(fr))
        nc.sync.dma_start(out=bqk_sb, in_=bqk.ap())
        nc.sync.dma_start(out=ident, in_=id128.ap().bitcast(fr))
        nc.sync.dma_start(out=ones_sb, in_=vones.ap().bitcast(fr)[0:1, 0:64])
        v3v = V3.rearrange("p c (h e) -> p c h e", e=DH + 1)
        nc.sync.dma_start(
            out=v3v[:, :, :, DH : DH + 1],
            in_=vones.ap().bitcast(fr)[:, 1:97].rearrange("p (c h) -> p c h", h=3)[
                :, :, :
            ].unsqueeze(3),
        )

        # ---- Phase 1+2: transpose x by row-chunks and project QKV ----
        with (
            tc.tile_pool(name="xs", bufs=2) as xsp,
            tc.tile_pool(name="xt", bufs=2) as xtp,
            tc.tile_pool(name="pst", bufs=2, space="PSUM") as pst,
            tc.tile_pool(name="psq", bufs=2, space="PSUM") as psq,
            tc.tile_pool(name="psv", bufs=2, space="PSUM") as psv,
        ):
            for r in range(S // RC):
                xs = xsp.tile([128, RC // 128, DM], fr)
                nc.sync.dma_start(
                    out=xs,
                    in_=xb.ap().bitcast(fr)[r * RC : (r + 1) * RC, :].rearrange(
                        "(c p) d -> p c d", p=128
                    ),
                )
                xt = xtp.tile([128, KCH, RC], fr)
                if 'p12' not in stages:
                    continue
                for k in range(KCH):
                    for c in range(RC // 128):
                        pt = pst.tile([128, 128], fr)
                        nc.tensor.transpose(
                            pt,
                            xs[:, c, 128 * k : 128 * (k + 1)],
                            ident,
                        )
                        nc.vector.tensor_copy(xt[:, k, 128 * c : 128 * (c + 1)], pt)
                # transposed Q/K products: 4 column chunks of 128
                for ch in range(4):
                    pq = psq.tile([128, RC], f32)
                    for k in range(KCH):
                        nc.tensor.matmul(
                            pq,
                            wt_sb[:, k, 128 * ch : 128 * (ch + 1)],
                            xt[:, k, :],
                            start=(k == 0),
                            stop=(k == KCH - 1),
                        )
                    nc.vector.tensor_scalar_add(
                        QK[ch][:, r * RC : (r + 1) * RC], pq, bqk_sb[:, ch : ch + 1]
                    )
                # V natural: rows on partitions
                for c in range(RC // 128):
                    pv = psv.tile([128, 192], f32)
                    for k in range(KCH):
                        nc.tensor.matmul(
                            pv,
                            xt[:, k, 128 * c : 128 * (c + 1)],
                            wv_sb[:, k, :],
                            start=(k == 0),
                            stop=(k == KCH - 1),
                        )
                    sidx = r * (RC // 128) + c
                    dst = V3[:, sidx : sidx + 1, :].rearrange(
                        "p c (h e) -> p (c h) e", e=DH + 1
                    )[:, :, 0:DH]
                    nc.vector.tensor_copy(dst, pv.rearrange("p (h e) -> p h e", e=DH))

        # ---- Phase 3: causal attention + partial out-projection ----
        # One-deep software pipeline: S matmuls for iteration i+1 are emitted
        # before iteration i's exp/PV so the in-order PE never idles on ACT.
        # Normalize+out-proj of q-block j-1 is emitted inside q-block j.
        with (
            tc.tile_pool(name="ptp", bufs=6) as ptp,
            tc.tile_pool(name="atp", bufs=2) as atp,
            tc.tile_pool(name="rcp", bufs=4) as rcp,
            tc.tile_pool(name="zsp", bufs=6) as zsp,
            tc.tile_pool(name="psS", bufs=4, space="PSUM") as psS,
            tc.tile_pool(name="psO", bufs=1, space="PSUM") as psO,
            tc.tile_pool(name="psZ", bufs=1, space="PSUM") as psZ,
        ):
            def emit_s(j, i):
                qs = slice(j * QB, (j + 1) * QB)
                ks = slice(128 * i, 128 * (i + 1))
                half = 64 * (i % 2)
                srcs = [
                    (QK[2][0:64, ks], QK[0][0:64, qs]),
                    (QK[2][64:128, ks], QK[0][64:128, qs]),
                    (QK[3][half : half + 64, ks], QK[1][half : half + 64, qs]),
                ]
                out = []
                for h, (kt, qt) in enumerate(srcs):
                    ps = psS.tile([128, QB], f32, tag="s", name="s")
                    nc.tensor.matmul(ps, kt, qt, start=True, stop=True)
                    out.append(ps)
                return out

            def emit_epv(j, i, s_tiles, os_, nkb):
                for h, ps in enumerate(s_tiles):
                    pt_t = ptp.tile([128, QB], fr, tag="pt", name="pt")
                    nc.scalar.activation(
                        pt_t, ps, mybir.ActivationFunctionType.Exp, scale=DH**-0.5
                    )
                    if i >= 4 * j:  # diagonal block: causal mask
                        o = 128 * i - QB * j
                        w = o + 128
                        nc.vector.tensor_mul(
                            pt_t[:, 0:w],
                            pt_t[:, 0:w],
                            mask_sb[:, 384 - o : 384 - o + w],
                        )
                    vsl = V3[:, i : i + 1, 65 * h : 65 * h + 65].rearrange(
                        "p a b -> p (a b)"
                    )
                    nc.tensor.matmul(
                        os_[h], vsl, pt_t, start=(i == 0), stop=(i == nkb - 1)
                    )

            def emit_tail(j, os_):
                at12 = atp.tile([128, QB], tag="a12", dtype=fr, name="at12")
                at2 = atp.tile([64, QB], tag="a2", dtype=fr, name="at2")
                dsts = [at12[0:64, :], at12[64:128, :], at2[0:64, :]]
                for h in range(3 if 'norm' in stages else 0):
                    rc1 = rcp.tile([1, QB], fr, name="rc1")
                    with nc.allow_low_precision(reason="fp32r softmax denom"):
                        nc.vector.reciprocal(rc1, os_[h][DH : DH + 1, :])
                    pb = psZ.tile([64, QB], f32, tag="zb", name="pb")
                    nc.tensor.matmul(pb, ones_sb, rc1, start=True, stop=True)
                    bb = rcp.tile([64, QB], f32, tag="bb", name="bb")
                    nc.vector.tensor_copy(bb, pb)
                    nc.vector.tensor_mul(dsts[h], os_[h][0:DH, :], bb)
                for m in range(QB // 128 if 'oproj' in stages else 0):
                    for n0, nw in ((0, 512), (512, 256)):
                        pz = psZ.tile([128, nw], f32, tag="zb", name="pz")
                        nc.tensor.matmul(
                            pz,
                            at12[:, 128 * m : 128 * (m + 1)],
                            wo1_sb[:, n0 : n0 + nw],
                            start=True,
                            stop=False,
                        )
                        nc.tensor.matmul(
                            pz,
                            at2[:, 128 * m : 128 * (m + 1)],
                            wo2_sb[:, n0 : n0 + nw],
                            start=False,
                            stop=True,
                        )
                        zs = zsp.tile([128, nw], f32, tag="zs", name="zs")
                        nc.vector.tensor_copy(zs, pz)
                        r0 = j * QB + 128 * m
                        nc.sync.dma_start(
                            out=z.ap()[r0 : r0 + 128, n0 : n0 + nw], in_=zs
                        )

            pending = None  # (j, os_) awaiting normalize+oproj emission
            for j in range(NQB if 'attn' in stages else 0):
                nkb = 4 * j + 4
                os_ = [
                    psO.tile([DH + 1, QB], f32, tag=f"o{h}", name=f"o{h}")
                    for h in range(3)
                ]
                if True:
                    prev_s = emit_s(j, 0)
                    for i in range(nkb):
                        if i + 1 < nkb:
                            nxt = emit_s(j, i + 1)
                        else:
                            nxt = None
                        emit_epv(j, i, prev_s, os_, nkb)
                        prev_s = nxt
                        if i == 1 and pending is not None:
                            emit_tail(*pending)
                            pending = None
                if pending is not None:
                    emit_tail(*pending)
                pending = (j, os_)
            if pending is not None:
                emit_tail(*pending)

    nc.compile()
    return nc


def _core_inputs(c, x, w_qkv, b_qkv, w_out, mk):
    b = c // 4
    h0 = 3 * (c % 4)

    def cols(mat_idx, h):
        base = mat_idx * DM + h * DH
        return w_qkv[:, base : base + DH]

    wt = np.ascontiguousarray(
        np.concatenate(
            [cols(0, h0), cols(0, h0 + 1), cols(0, h0 + 2), cols(0, h0 + 2),
             cols(1, h0), cols(1, h0 + 1), cols(1, h0 + 2), cols(1, h0 + 2)],
            axis=1,
        )
    )
    wv = np.ascontiguousarray(
        np.concatenate([cols(2, h0), cols(2, h0 + 1), cols(2, h0 + 2)], axis=1)
    )
    wo = np.ascontiguousarray(w_out[h0 * DH : h0 * DH + 192, :])

    def bias(mat_idx, h):
        base = mat_idx * DM + h * DH
        return b_qkv[base : base + DH]

    bqk = np.stack(
        [
            np.concatenate([bias(0, h0), bias(0, h0 + 1)]),
            np.concatenate([bias(0, h0 + 2), bias(0, h0 + 2)]),
            np.concatenate([bias(1, h0), bias(1, h0 + 1)]),
            np.concatenate([bias(1, h0 + 2), bias(1, h0 + 2)]),
        ],
        axis=1,
    ).astype(np.float32)
    return {
        "xb": np.ascontiguousarray(x[b]),
        "wt": wt,
        "wv": wv,
        "wo": wo,
        "mk": mk,
        "bqk": np.ascontiguousarray(bqk),
        "id128": np.eye(128, dtype=np.float32),
        "vones": np.ones((128, 97), dtype=np.float32),
    }


def _run(inputs, trace=False):
    from concourse.bass_utils import run_bass_kernel_spmd

    x = np.asarray(inputs["x"], dtype=np.float32)
    w_qkv = np.asarray(inputs["w_qkv"], dtype=np.float32)
    b_qkv = np.asarray(inputs["b_qkv"], dtype=np.float32)
    w_out = np.asarray(inputs["w_out"], dtype=np.float32)
    b_out = np.asarray(inputs["b_out"], dtype=np.float32)

    if "nc" not in _cache:
        _cache["nc"] = _build()
    nc = _cache["nc"]

    mk = (
        np.arange(896, dtype=np.int64)[None, :]
        >= (np.arange(128, dtype=np.int64)[:, None] + 384)
    ).astype(np.float32)
    in_maps = [_core_inputs(c, x, w_qkv, b_qkv, w_out, mk) for c in range(NCORES)]
    res = run_bass_kernel_spmd(
        nc, in_maps, core_ids=list(range(NCORES)), trace=trace
    )

    out = np.zeros((B, S, DM), dtype=np.float32)
    for c in range(NCORES):
        out[c // 4] += res.results[c]["z"]
    out += b_qkv[2 * DM : 3 * DM] @ w_out + b_out
    return out, res


def kernel(**inputs):
    out, _ = _run(inputs, trace=False)
    return out


# revision 21
# speedup vs baseline: 2.2752x; 1.0316x over previous
"""Multi-head causal attention (B=2, S=4096, DM=768, H=12) on 8 trn2 cores.

Sharding: core c handles batch c//4, heads 3*(c%4) .. +3 (data + head parallel).
Each core computes QKV projection for its 3 heads, causal flash-style attention,
and a partial output projection (contraction over its 192 out-dims). Host sums
the 4 partials per batch; qkv/out biases are folded in exactly on the host.

On-chip layout: Q,K are kept transposed [dh, S] (computed via PE-transpose of x
feeding the QKV matmul); scores are computed as S^T blocks [sk=128, sq=512] so
softmax needs no partition-dim reductions: the running denominator comes from an
appended ones-column in V, and the final divide is broadcast with a K=1 matmul.
No max-subtraction is needed (scores ~ N(0,1) in fp32). All matmuls use fp32r.
"""
import sys

sys.path.insert(0, "/opt/trn_rl_repo")
import numpy as np

B, S, DM, H, DH = 2, 4096, 768, 12, 64
NCORES = 8
QB = 512          # q-block (psum free dim)
KB = 128          # k-block (psum partition dim)
NQB = S // QB     # 8
RC = 512          # row chunk for the projection phase
KCH = DM // 128   # 6 contraction chunks of the model dim

_cache = {}


def _build(loop_n=None, stages=frozenset({'p12','attn','norm','oproj'})):
    import concourse.mybir as mybir
    import concourse.tile as tile
    from concourse import bacc
    from contextlib import ExitStack

    f32 = mybir.dt.float32
    fr = mybir.dt.float32r

    nc = bacc.Bacc("TRN2", target_bir_lowering=False, debug=False)
    xb = nc.dram_tensor("xb", [S, DM], f32, kind="ExternalInput")
    wt = nc.dram_tensor("wt", [DM, 512], f32, kind="ExternalInput")
    wv = nc.dram_tensor("wv", [DM, 192], f32, kind="ExternalInput")
    wo = nc.dram_tensor("wo", [192, DM], f32, kind="ExternalInput")
    mk = nc.dram_tensor("mk", [128, 896], f32, kind="ExternalInput")
    bqk = nc.dram_tensor("bqk", [128, 4], f32, kind="ExternalInput")
    id128 = nc.dram_tensor("id128", [128, 128], f32, kind="ExternalInput")
    vones = nc.dram_tensor("vones", [128, 97], f32, kind="ExternalInput")
    z = nc.dram_tensor("z", [S, DM], f32, kind="ExternalOutput")

    with tile.TileContext(nc) as tc, ExitStack() as st:
        if loop_n is not None:
            st.enter_context(tc.For_i(0, loop_n, 1))
        pers = st.enter_context(tc.tile_pool(name="pers", bufs=1))
        # 0: Q^T heads 0|1 packed in partition halves, 1: Q^T head2 duplicated,
        # 2: K^T heads 0|1, 3: K^T head2 duplicated
        QK = [
            pers.tile([128, S], fr, tag=f"qk{i}", name=f"qk{i}") for i in range(4)
        ]
        # V natural layout + ones column per head: [sk%128, sk//128, 3*(64+1)]
        V3 = pers.tile([128, S // 128, 3 * (DH + 1)], fr)
        wt_sb = pers.tile([128, KCH, 512], fr)
        wv_sb = pers.tile([128, KCH, 192], fr)
        wo1_sb = pers.tile([128, DM], fr)
        wo2_sb = pers.tile([64, DM], fr)
        mask_sb = pers.tile([128, 896], fr)
        bqk_sb = pers.tile([128, 4], f32)
        ident = pers.tile([128, 128], fr)
        ones_sb = pers.tile([1, 64], fr)

        nc.sync.dma_start(out=wt_sb, in_=wt.ap().bitcast(fr).rearrange("(h p) c -> p h c", p=128))
        nc.sync.dma_start(out=wv_sb, in_=wv.ap().bitcast(fr).rearrange("(h p) c -> p h c", p=128))
        nc.sync.dma_start(out=wo1_sb, in_=wo.ap().bitcast(fr)[0:128, :])
        nc.sync.dma_start(out=wo2_sb, in_=wo.ap().bitcast(fr)[128:192, :])
        nc.sync.dma_start(out=mask_sb, in_=mk.ap().bitcast(fr))
        nc.sync.dma_start(out=bqk_sb, in_=bqk.ap())
        nc.sync.dma_start(out=ident, in_=id128.ap().bitcast(fr))
        nc.sync.dma_start(out=ones_sb, in_=vones.ap().bitcast(fr)[0:1, 0:64])
        v3v = V3.rearrange("p c (h e) -> p c h e", e=DH + 1)
        nc.sync.dma_start(
            out=v3v[:, :, :, DH : DH + 1],
            in_=vones.ap().bitcast(fr)[:, 1:97].rearrange("p (c h) -> p c h", h=3)[
                :, :, :
            ].unsqueeze(3),
        )

        # ---- Phase 1+2: transpose x by row-chunks and project QKV ----
        with (
            tc.tile_pool(name="xs", bufs=2) as xsp,
            tc.tile_pool(name="xt", bufs=2) as xtp,
            tc.tile_pool(name="pst", bufs=2, space="PSUM") as pst,
            tc.tile_pool(name="psq", bufs=2, space="PSUM") as psq,
            tc.tile_pool(name="psv", bufs=2, space="PSUM") as psv,
        ):
            for r in range(S // RC):
                xs = xsp.tile([128, RC // 128, DM], fr)
                nc.sync.dma_start(
                    out=xs,
                    in_=xb.ap().bitcast(fr)[r * RC : (r + 1) * RC, :].rearrange(
                        "(c p) d -> p c d", p=128
                    ),
                )
                xt = xtp.tile([128, KCH, RC], fr)
                if 'p12' not in stages:
                    continue
                for k in range(KCH):
                    for c in range(RC // 128):
                        pt = pst.tile([128, 128], fr)
                        nc.tensor.transpose(
                            pt,
                            xs[:, c, 128 * k : 128 * (k + 1)],
                            ident,
                        )
                        nc.vector.tensor_copy(xt[:, k, 128 * c : 128 * (c + 1)], pt)
                # transposed Q/K products: 4 column chunks of 128
                for ch in range(4):
                    pq = psq.tile([128, RC], f32)
                    for k in range(KCH):
                        nc.tensor.matmul(
                            pq,
                            wt_sb[:, k, 128 * ch : 128 * (ch + 1)],
                            xt[:, k, :],
                            start=(k == 0),
                            stop=(k == KCH - 1),
                        )
                    nc.vector.tensor_scalar_add(
                        QK[ch][:, r * RC : (r + 1) * RC], pq, bqk_sb[:, ch : ch + 1]
                    )
                # V natural: rows on partitions
                for c in range(RC // 128):
                    pv = psv.tile([128, 192], f32)
                    for k in range(KCH):
                        nc.tensor.matmul(
                            pv,
                            xt[:, k, 128 * c : 128 * (c + 1)],
                            wv_sb[:, k, :],
                            start=(k == 0),
                            stop=(k == KCH - 1),
                        )
                    sidx = r * (RC // 128) + c
                    dst = V3[:, sidx : sidx + 1, :].rearrange(
                        "p c (h e) -> p (c h) e", e=DH + 1
                    )[:, :, 0:DH]
                    nc.vector.tensor_copy(dst, pv.rearrange("p (h e) -> p h e", e=DH))

        # ---- Phase 3: causal attention + partial out-projection ----
        # Super-blocks: two 128-row k-blocks share one [128,1024] psum tile
        # (2 banks) -> one exp per super. Heads run in two waves per q-block
        # (h0+h1 paired on PE row groups, then h2 row-pair packed). S psum is
        # released by the exp (PV reads the SBUF PT), so shallow buffering
        # works. PSUM: sa0 2x2 + sa1 1x2 + o 2x1 = 8 banks.
        with (
            tc.tile_pool(name="ptp", bufs=4) as ptp,
            tc.tile_pool(name="atp", bufs=2) as atp,
            tc.tile_pool(name="rcp", bufs=4) as rcp,
            tc.tile_pool(name="zsp", bufs=6) as zsp,
            tc.tile_pool(name="drp", bufs=3, space="DRAM") as drp,
            tc.tile_pool(name="psS", bufs=1, space="PSUM") as psS,
            tc.tile_pool(name="psO", bufs=2, space="PSUM") as psO,
        ):
            def emit_sup(j, s, h):
                """S^T matmuls for super s (k-blocks 2s, 2s+1) of head h."""
                qs = slice(j * QB, (j + 1) * QB)
                sup = psS.tile(
                    [128, 2 * QB], f32, tag="sa1" if h == 1 else "sa0",
                    name="sup", bufs=1 if h == 1 else 2,
                )
                for half, i in ((0, 2 * s), (1, 2 * s + 1)):
                    ks = slice(128 * i, 128 * (i + 1))
                    if h == 2:
                        rows = slice(64 * (i % 2), 64 * (i % 2) + 64)
                        kt, qt = QK[3][rows, ks], QK[1][rows, qs]
                    else:
                        rows = slice(64 * h, 64 * h + 64)
                        kt, qt = QK[2][rows, ks], QK[0][rows, qs]
                    nc.tensor.matmul(
                        sup[:, half * QB : (half + 1) * QB], kt, qt,
                        start=True, stop=True,
                    )
                return sup

            def emit_epv(j, s, h, sup, o_t, nsup):
                pt_t = ptp.tile([128, 2 * QB], fr, tag="pt", name="pt")
                nc.scalar.activation(
                    pt_t, sup, mybir.ActivationFunctionType.Exp, scale=DH**-0.5
                )
                for half, i in ((0, 2 * s), (1, 2 * s + 1)):
                    col = half * QB
                    if i >= 4 * j:  # diagonal block: causal mask
                        o = 128 * i - QB * j
                        if o > 0:  # cols [0,o) are fully masked; mask_sb's
                            # first 384 cols are all zeros - copy zeros in
                            nc.vector.tensor_copy(
                                pt_t[:, col : col + o], mask_sb[:, 0:o]
                            )
                        nc.vector.tensor_mul(
                            pt_t[:, col + o : col + o + 128],
                            pt_t[:, col + o : col + o + 128],
                            mask_sb[:, 384:512],
                        )
                    vsl = V3[:, i : i + 1, 65 * h : 65 * h + 65].rearrange(
                        "p a b -> p (a b)"
                    )
                    nc.tensor.matmul(
                        o_t,
                        vsl,
                        pt_t[:, col : col + QB],
                        start=(i == 0),
                        stop=(i == 2 * nsup - 1),
                    )

            def emit_norm(h, o_t, at12, at2):
                if 'norm' not in stages:
                    return
                dsts = [at12[0:64, :], at12[64:128, :], at2[0:64, :]]
                rc1 = rcp.tile([1, QB], fr, name="rc1")
                with nc.allow_low_precision(reason="fp32r softmax denom"):
                    nc.vector.reciprocal(rc1, o_t[DH : DH + 1, :])
                rcd = drp.tile([1, QB], fr, name="rcd")
                nc.sync.dma_start(out=rcd, in_=rc1)
                bb = rcp.tile([DH, QB], fr, tag="bb", name="bb")
                nc.sync.dma_start(out=bb, in_=rcd.partition_broadcast(DH))
                nc.vector.tensor_mul(dsts[h], o_t[0:DH, :], bb)

            def emit_oproj(j, at12, at2):
                for m in range(QB // 128 if 'oproj' in stages else 0):
                    for n0, nw in ((0, 512), (512, 256)):
                        pz = psS.tile(
                            [128, nw], f32, tag="sa0" if m % 2 else "sa1",
                            name="pz", bufs=2 if m % 2 else 1,
                        )
                        nc.tensor.matmul(
                            pz,
                            at12[:, 128 * m : 128 * (m + 1)],
                            wo1_sb[:, n0 : n0 + nw],
                            start=True,
                            stop=False,
                        )
                        nc.tensor.matmul(
                            pz,
                            at2[:, 128 * m : 128 * (m + 1)],
                            wo2_sb[:, n0 : n0 + nw],
                            start=False,
                            stop=True,
                        )
                        zs = zsp.tile([128, nw], f32, tag="zs", name="zs")
                        nc.vector.tensor_copy(zs, pz)
                        r0 = j * QB + 128 * m
                        nc.sync.dma_start(
                            out=z.ap()[r0 : r0 + 128, n0 : n0 + nw], in_=zs
                        )

            for j in range(NQB if 'attn' in stages else 0):
                nsup = 2 * j + 2
                at12 = atp.tile([128, QB], tag="a12", dtype=fr, name="at12")
                at2 = atp.tile([64, QB], tag="a2", dtype=fr, name="at2")
                # wave A: heads 0 and 1 on PE row groups 0/1
                o0 = psO.tile([DH + 1, QB], f32, tag="o", name="o0")
                o1 = psO.tile([DH + 1, QB], f32, tag="o", name="o1")
                cur = {0: emit_sup(j, 0, 0), 1: emit_sup(j, 0, 1)}
                for s in range(nsup):
                    nxt = {}
                    if s + 1 < nsup:
                        nxt[0] = emit_sup(j, s + 1, 0)
                    emit_epv(j, s, 0, cur[0], o0, nsup)
                    if s + 1 < nsup:
                        nxt[1] = emit_sup(j, s + 1, 1)
                    emit_epv(j, s, 1, cur[1], o1, nsup)
                    cur = nxt
                emit_norm(0, o0, at12, at2)
                emit_norm(1, o1, at12, at2)
                # wave B: head 2 row-pair packed
                o2 = psO.tile([DH + 1, QB], f32, tag="o", name="o2")
                prev = emit_sup(j, 0, 2)
                for s in range(nsup):
                    nxt = emit_sup(j, s + 1, 2) if s + 1 < nsup else None
                    emit_epv(j, s, 2, prev, o2, nsup)
                    prev = nxt
                emit_norm(2, o2, at12, at2)
                emit_oproj(j, at12, at2)

    nc.compile()
    return nc


def _core_inputs(c, x, w_qkv, b_qkv, w_out, mk):
    b = c // 4
    h0 = 3 * (c % 4)

    def cols(mat_idx, h):
        base = mat_idx * DM + h * DH
        return w_qkv[:, base : base + DH]

    wt = np.ascontiguousarray(
        np.concatenate(
            [cols(0, h0), cols(0, h0 + 1), cols(0, h0 + 2), cols(0, h0 + 2),
             cols(1, h0), cols(1, h0 + 1), cols(1, h0 + 2), cols(1, h0 + 2)],
            axis=1,
        )
    )
    wv = np.ascontiguousarray(
        np.concatenate([cols(2, h0), cols(2, h0 + 1), cols(2, h0 + 2)], axis=1)
    )
    wo = np.ascontiguousarray(w_out[h0 * DH : h0 * DH + 192, :])

    def bias(mat_idx, h):
        base = mat_idx * DM + h * DH
        return b_qkv[base : base + DH]

    bqk = np.stack(
        [
            np.concatenate([bias(0, h0), bias(0, h0 + 1)]),
            np.concatenate([bias(0, h0 + 2), bias(0, h0 + 2)]),
            np.concatenate([bias(1, h0), bias(1, h0 + 1)]),
            np.concatenate([bias(1, h0 + 2), bias(1, h0 + 2)]),
        ],
        axis=1,
    ).astype(np.float32)
    return {
        "xb": np.ascontiguousarray(x[b]),
        "wt": wt,
        "wv": wv,
        "wo": wo,
        "mk": mk,
        "bqk": np.ascontiguousarray(bqk),
        "id128": np.eye(128, dtype=np.float32),
        "vones": np.ones((128, 97), dtype=np.float32),
    }


def _run(inputs, trace=False):
    from concourse.bass_utils import run_bass_kernel_spmd

    x = np.asarray(inputs["x"], dtype=np.float32)
    w_qkv = np.asarray(inputs["w_qkv"], dtype=np.float32)
    b_qkv = np.asarray(inputs["b_qkv"], dtype=np.float32)
    w_out = np.asarray(inputs["w_out"], dtype=np.float32)
    b_out = np.asarray(inputs["b_out"], dtype=np.float32)

    if "nc" not in _cache:
        _cache["nc"] = _build()
    nc = _cache["nc"]

    mk = (
        np.arange(896, dtype=np.int64)[None, :]
        >= (np.arange(128, dtype=np.int64)[:, None] + 384)
    ).astype(np.float32)
    in_maps = [_core_inputs(c, x, w_qkv, b_qkv, w_out, mk) for c in range(NCORES)]
    res = run_bass_kernel_spmd(
        nc, in_maps, core_ids=list(range(NCORES)), trace=trace
    )

    out = np.zeros((B, S, DM), dtype=np.float32)
    for c in range(NCORES):
        out[c // 4] += res.results[c]["z"]
    out += b_qkv[2 * DM : 3 * DM] @ w_out + b_out
    return out, res


def kernel(**inputs):
    out, _ = _run(inputs, trace=False)
    return out
